# revision 8
# baseline (speedup 1.0000x reference)
"""Trainium2 Bass kernel for nn_Channel_Seq_Big_Attention.

Reference computation (per batch b of 8, fully data-parallel across 8 cores):
  x: (N=128, M=8, D=512) tokens; q = x@w_q, k,v = x@w_kv (INNER=512, H=8, DH=64)
  sim[i,j,m,z] = q[i,m]·k[j,z] * DH**-0.5     (cross-seq, cross-modality)
  attn = softmax over the QUERY-seq dim i (axis 2 of (b,h,i,j,m,z))
  out[i,z,d] = sum_{j,m} attn[i,j,m,z] v[j,m,d]
  y = out.reshape(N, M*H*DH) @ w_out + b_out   (col order z*H*DH + h*DH + d)

Because softmax normalizes over i (not the contracted j), the denominator
L[j,m,z] = sum_i exp(S[i,j,m,z]) folds into V:
  out_z = sum_m exp(S_mz) @ (v_m / L_mz[j])
Device dataflow per core (tokens kept in modality-major order m*N+j):
  - host pre-transposes/casts x to xT (D, T) bf16; projections contract D on
    partitions producing qT/kT ((h dh), T) directly.
  - S^T tiles (keys j on partitions, queries (m,i) free) via matmuls into
    single-bank PSUM tiles (one per (head, token-half); fine granularity so
    the recycle wait never reorders the head pair apart) -> exp on ScalarE
    (PSUM->SBUF bf16) -> segmented row sums split VectorE/GpSimd ->
    reciprocal -> scale v by Linv (per-partition j scalars) on GpSimd ->
    PV matmuls (lhsT=v-scaled, rhs=exp(S^T)) accumulate over m in PSUM.
    PV lags the sim/exp chain by THREE z-steps (the chain latency spans
    ~2.5 steps; a shorter lag head-of-line blocks the PE queue).
  - out projection streamed from HBM as 32 column-tiles [8x128 rows x 512
    cols]: each tile is consumed by exactly ONE partial-projection group,
    so a pool slot recycles every z-step and the DMA engines stream w_out
    continuously instead of bursting at pair boundaries.  Final pair's
    partials run in a dense tail with per-block stores overlapping the
    matmuls.
"""

import sys

import numpy as np

for _p in ("/opt/trn_rl_repo",):
    if _p not in sys.path:
        sys.path.insert(0, _p)

import os  # noqa: E402

import ml_dtypes  # noqa: E402

USE_POOL = os.environ.get("K_POOL", "0") == "1"

B, N, M, D = 8, 128, 8, 512
H, DH = 8, 64
INNER = H * DH          # 512
T = N * M               # 1024 tokens per batch element
CD = INNER * M          # 4096 contraction dim of out projection
NCORES = 8

BF16 = ml_dtypes.bfloat16

_CACHE = {}


def build_nc():
    import concourse.bass as bass
    import concourse.mybir as mybir
    import concourse.tile as tile
    from concourse import bacc

    fp32 = mybir.dt.float32
    bf16 = mybir.dt.bfloat16

    nc = bacc.Bacc(trn_type="TRN2", target_bir_lowering=False, debug=False)

    xT = nc.dram_tensor("xT", (D, T), bf16, kind="ExternalInput").ap()
    w_q = nc.dram_tensor("w_q", (D, INNER), bf16, kind="ExternalInput").ap()
    w_k = nc.dram_tensor("w_k", (D, INNER), bf16, kind="ExternalInput").ap()
    w_v = nc.dram_tensor("w_v", (D, INNER), bf16, kind="ExternalInput").ap()
    w_out = nc.dram_tensor("w_out", (CD, CD), bf16, kind="ExternalInput").ap()
    y = nc.dram_tensor("y", (N, CD), fp32, kind="ExternalOutput").ap()

    KC = D // 128        # 4 contraction chunks for the projections
    PC = INNER // 128    # 4 partition chunks of qT/kT
    SCALE = DH ** -0.5
    NKC = CD // 128      # 32 contraction chunks of the out projection
    PV_LAG = 3
    # w_out rows r = idx*512 + gp*128 + p: column-tile (gp, nb) gathers the
    # 8 idx-chunks of contraction group gp for output columns nb*512..
    wo_src = w_out.rearrange("(i g p) c -> g p i c", i=M, g=4)

    with tile.TileContext(nc) as tc:
        with tc.tile_pool(name="persist", bufs=1) as persist:
            qT_sb = persist.tile([128, PC, T], bf16)
            kT_sb = persist.tile([128, PC, T], bf16)
            v_sb = persist.tile([128, M, INNER], bf16)
            ofT_sb = persist.tile([128, NKC, N], bf16)
            y_sb = persist.tile([128, CD], fp32)
            xT_sb = persist.tile([128, KC, T], bf16)
            wq_sb = persist.tile([128, KC, INNER], bf16)
            wk_sb = persist.tile([128, KC, INNER], bf16)
            wv_sb = persist.tile([128, KC, INNER], bf16)

            for kc in range(KC):
                nc.sync.dma_start(wq_sb[:, kc, :], w_q[kc * 128:(kc + 1) * 128, :])
                nc.sync.dma_start(xT_sb[:, kc, :], xT[kc * 128:(kc + 1) * 128, :])
            for kc in range(KC):
                nc.sync.dma_start(wk_sb[:, kc, :], w_k[kc * 128:(kc + 1) * 128, :])
            for kc in range(KC):
                nc.sync.dma_start(wv_sb[:, kc, :], w_v[kc * 128:(kc + 1) * 128, :])

            wo_pool = tc.alloc_tile_pool(name="wo_pool", bufs=12)
            wo_tiles = {}

            # ---- projections: qT/kT ((h dh) on partitions, tokens free), v ----
            with tc.tile_pool(name="proj_psum", bufs=4, space="PSUM") as proj_psum:
                for dst, w_sb in ((qT_sb, wq_sb), (kT_sb, wk_sb)):
                    for pc in range(PC):
                        for th in range(T // 512):
                            pj = proj_psum.tile([128, 512], fp32, name="pj", tag="pj")
                            for kc in range(KC):
                                nc.tensor.matmul(
                                    pj[:],
                                    w_sb[:, kc, pc * 128:(pc + 1) * 128],
                                    xT_sb[:, kc, th * 512:(th + 1) * 512],
                                    start=(kc == 0),
                                    stop=(kc == KC - 1),
                                )
                            nc.scalar.copy(dst[:, pc, th * 512:(th + 1) * 512], pj[:])
                for m in range(M):
                    pj = proj_psum.tile([128, 512], fp32, name="pj", tag="pj")
                    for kc in range(KC):
                        nc.tensor.matmul(
                            pj[:],
                            xT_sb[:, kc, m * 128:(m + 1) * 128],
                            wv_sb[:, kc, :],
                            start=(kc == 0),
                            stop=(kc == KC - 1),
                        )
                    if USE_POOL:
                        # store v/128: cancels the avg-pool's 1/128 exactly
                        nc.scalar.activation(
                            v_sb[:, m, :], pj[:],
                            mybir.ActivationFunctionType.Copy, scale=1.0 / 128,
                        )
                    else:
                        nc.scalar.copy(v_sb[:, m, :], pj[:])

            # w_out column-tile stream, issued after the projection trace so
            # the input loads own the DMA engines for the first few us.
            # Issue order == consumption order ((gp, nb) lexicographic).
            for gp in range(4):
                for nb in range(M):
                    wo_t = wo_pool.tile([128, M, 512], bf16, name="wo_t", tag="wo")
                    nc.sync.dma_start(
                        wo_t[:], wo_src[gp, :, :, nb * 512:(nb + 1) * 512]
                    )
                    wo_tiles[(gp, nb)] = wo_t

            # ---- attention ----
            # Heads are processed in pairs (2g, 2g+1).  The two heads' sim
            # matmuls use K row-groups 0-63 / 64-127 and their PV matmuls use
            # output col-groups 0-63 / 64-127, so interleaving them lets the
            # PE array run both concurrently.
            with (
                tc.tile_pool(name="sim_psum", bufs=4, space="PSUM") as sim_psum,
                tc.tile_pool(name="pv_psum", bufs=1, space="PSUM") as pv_psum,
                tc.tile_pool(name="yp_psum", bufs=2, space="PSUM") as yp_psum,
                tc.tile_pool(name="p_pool", bufs=5) as p_pool,
                tc.tile_pool(name="vt_pool", bufs=6) as vt_pool,
                tc.tile_pool(name="stat_pool", bufs=10) as stat_pool,
            ):
                def emit_y_mms(gp, nb):
                    # partial out-projection matmuls: pair gp's 8 contraction
                    # chunks into output columns [nb*512, (nb+1)*512).
                    yp = yp_psum.tile([128, 512], fp32, name="yp", tag="yp")
                    wo_t = wo_tiles[(gp, nb)]
                    for idx in range(M):
                        kc = 4 * idx + gp
                        nc.tensor.matmul(
                            yp[:],
                            ofT_sb[:, kc, :],
                            wo_t[:, idx, :],
                            start=(idx == 0),
                            stop=(idx == M - 1),
                        )
                    return yp

                def emit_y_add(gp, nb, yp):
                    # VectorE accumulate, traced after the z-step's own DVE
                    # work to keep the DVE queue acyclic.
                    ysl = y_sb[:, nb * 512:(nb + 1) * 512]
                    if gp == 0:
                        nc.vector.tensor_copy(ysl, yp[:])
                    else:
                        nc.vector.tensor_tensor(
                            ysl, yp[:], ysl, op=mybir.AluOpType.add,
                        )

                for g in range(H // 2):  # head pairs
                    opv = pv_psum.tile([128, M * 128], fp32, name="opv", tag="opv")
                    pv_queue = []

                    def emit_pv(zz, p_z, vt_z, opv=opv):
                        # one accumulation group per head per z-region (groups
                        # in a PSUM zero region must not interleave start/stop)
                        for hh in range(2):
                            for m in range(M):
                                nc.tensor.matmul(
                                    opv[hh * 64:hh * 64 + 64, bass.ts(zz, 128)],
                                    vt_z[:, m, hh, :],
                                    p_z[:, hh * T + m * 128:hh * T + (m + 1) * 128],
                                    start=(m == 0),
                                    stop=(m == M - 1),
                                )
                    hc = g
                    qh = (qT_sb[0:64, hc, :], qT_sb[64:128, hc, :])
                    kh = (kT_sb[0:64, hc, :], kT_sb[64:128, hc, :])
                    for z in range(M):
                        # S^T_z per head: keys (z,j) on partitions, (m,i)
                        # free.  One single-bank PSUM tile per (th, head),
                        # exp'd individually: a tile recycles as soon as its
                        # own exp ran, so the head-pair matmuls stay adjacent
                        # in the PE queue and overlap on distinct row groups.
                        p_sb = p_pool.tile([128, 2 * T], bf16, name="p_sb", tag="p")
                        pieces = []
                        for th in range(T // 512):
                            for hh in range(2):
                                ps = sim_psum.tile([128, 512], fp32, name="ps", tag="ps")
                                nc.tensor.matmul(
                                    ps[:], kh[hh][:, bass.ts(z, 128)],
                                    qh[hh][:, bass.ts(th, 512)],
                                    start=True, stop=True,
                                )
                                pieces.append((ps, hh * T + th * 512))
                        # PV for z-PV_LAG (its exp/stats/vt chain is
                        # complete) goes right after sim, so ready work never
                        # queues behind out-proj matmuls that may wait on the
                        # w_out stream.
                        if len(pv_queue) >= PV_LAG:
                            emit_pv(*pv_queue.pop(0))
                        yp_fill = emit_y_mms(g - 1, z) if g > 0 else None
                        for ps, off in pieces:
                            nc.scalar.activation(
                                p_sb[:, off:off + 512], ps[:],
                                mybir.ActivationFunctionType.Exp, scale=SCALE,
                            )
                        # L[j, (h, m)] = sum_i P^T[j, (h, m, i)] on VectorE.
                        # USE_POOL: avg-pool instead of tensor_reduce (the
                        # 1/128 is pre-folded into v at projection time).
                        lsum = stat_pool.tile([128, 2 * M], fp32, name="lsum", tag="ls")
                        pv3 = p_sb[:].rearrange("p (hm i) -> p hm i", i=128)
                        if USE_POOL:
                            nc.vector.pool_avg(lsum[:], pv3)
                        else:
                            nc.vector.tensor_reduce(
                                lsum[:], pv3,
                                axis=mybir.AxisListType.X, op=mybir.AluOpType.add,
                            )
                        linv = stat_pool.tile([128, 2 * M], fp32, name="linv", tag="li")
                        nc.vector.reciprocal(linv[:], lsum[:])
                        if yp_fill is not None:
                            emit_y_add(g - 1, z, yp_fill)
                        # vt[j, m, h, d] = v[j, m, (pair cols)] * Linv[j, (h, m)]
                        vt = vt_pool.tile([128, M, 2, DH], bf16, name="vt", tag="vt")
                        nc.gpsimd.tensor_tensor(
                            vt[:],
                            v_sb[:, :, g * 128:(g + 1) * 128].rearrange(
                                "p m (h d) -> p m h d", h=2
                            ),
                            linv[:].rearrange("p (h m) -> p m h", h=2)
                            .unsqueeze(3).broadcast_to((128, M, 2, DH)),
                            op=mybir.AluOpType.mult,
                        )
                        pv_queue.append((z, p_sb, vt))
                    for pv in pv_queue:  # flush the lagged z's of the pair
                        emit_pv(*pv)
                    # opv -> ofT split across Scalar/Vector so neither engine
                    # adds a full ~1.1us serial bubble at the pair boundary.
                    nc.scalar.copy(
                        ofT_sb[:, g:g + 16:4, :],
                        opv[:, 0:512].rearrange("p (z i) -> p z i", i=128),
                    )
                    nc.vector.tensor_copy(
                        ofT_sb[:, g + 16::4, :],
                        opv[:, 512:].rearrange("p (z i) -> p z i", i=128),
                    )

            # ---- last pair's out-projection partials + store ----
            with tc.tile_pool(name="ylast_psum", bufs=2, space="PSUM") as yp_psum2:
                def emit_y_last(nb):
                    yp = yp_psum2.tile([128, 512], fp32, name="yl", tag="yl")
                    wo_t = wo_tiles[(3, nb)]
                    for idx in range(M):
                        kc = 4 * idx + 3
                        nc.tensor.matmul(
                            yp[:],
                            ofT_sb[:, kc, :],
                            wo_t[:, idx, :],
                            start=(idx == 0),
                            stop=(idx == M - 1),
                        )
                    ysl = y_sb[:, nb * 512:(nb + 1) * 512]
                    nc.vector.tensor_tensor(ysl, yp[:], ysl, op=mybir.AluOpType.add)
                    nc.sync.dma_start(y[:, nb * 512:(nb + 1) * 512], ysl)

                for nb in range(CD // 512):
                    emit_y_last(nb)
            wo_pool.release()

    nc.compile()
    return nc


def _get_nc():
    if "nc" not in _CACHE:
        _CACHE["nc"] = build_nc()
    return _CACHE["nc"]


def _host_prep(x, w_q, w_kv, w_out):
    w_k = np.ascontiguousarray(w_kv[:, :INNER]).astype(BF16)
    w_v = np.ascontiguousarray(w_kv[:, INNER:]).astype(BF16)
    wq16 = np.ascontiguousarray(w_q).astype(BF16)
    wo16 = np.ascontiguousarray(w_out).astype(BF16)
    in_maps = []
    for b in range(B):
        # tokens modality-major: (M, N, D) -> (T, D); transpose to (D, T)
        xb = x[b].transpose(1, 0, 2).reshape(T, D)
        xT = np.ascontiguousarray(xb.T).astype(BF16)
        in_maps.append(
            {"xT": xT, "w_q": wq16, "w_k": w_k, "w_v": w_v, "w_out": wo16}
        )
    return in_maps


def kernel(x, w_q, w_kv, w_out, b_out):
    from concourse.bass_utils import run_bass_kernel_spmd

    nc = _get_nc()
    in_maps = _host_prep(
        np.asarray(x, np.float32),
        np.asarray(w_q, np.float32),
        np.asarray(w_kv, np.float32),
        np.asarray(w_out, np.float32),
    )
    res = run_bass_kernel_spmd(nc, in_maps, core_ids=list(range(NCORES)))
    ys = np.stack([res.results[c]["y"] for c in range(NCORES)], axis=0)
    ys = ys + np.asarray(b_out, np.float32)[None, None, :]
    return ys.reshape(B, N, M, D).astype(np.float32)


# revision 17
# speedup vs baseline: 1.2567x; 1.2567x over previous
"""Trainium2 Bass kernel for nn_Channel_Seq_Big_Attention.

Reference computation (per batch b of 8, fully data-parallel across 8 cores):
  x: (N=128, M=8, D=512) tokens; q = x@w_q, k,v = x@w_kv (INNER=512, H=8, DH=64)
  sim[i,j,m,z] = q[i,m]·k[j,z] * DH**-0.5     (cross-seq, cross-modality)
  attn = softmax over the QUERY-seq dim i (axis 2 of (b,h,i,j,m,z))
  out[i,z,d] = sum_{j,m} attn[i,j,m,z] v[j,m,d]
  y = out.reshape(N, M*H*DH) @ w_out + b_out   (col order z*H*DH + h*DH + d)

Because softmax normalizes over i (not the contracted j), the denominator
L[j,m,z] = sum_i exp(S[i,j,m,z]) folds into V:
  out_z = sum_m exp(S_mz) @ (v_m / L_mz[j])
Device dataflow per core (tokens kept in modality-major order m*N+j):
  - host pre-transposes/casts x to xT (D, T) bf16; projections contract D on
    partitions producing qT/kT ((h dh), T) directly.
  - S^T tiles (keys j on partitions, queries (m,i) free) via matmuls into
    single-bank PSUM tiles (one per (head, token-half); fine granularity so
    the recycle wait never reorders the head pair apart) -> exp on ScalarE
    (PSUM->SBUF bf16) -> segmented row sums split VectorE/GpSimd ->
    reciprocal -> scale v by Linv (per-partition j scalars) on GpSimd ->
    PV matmuls (lhsT=v-scaled, rhs=exp(S^T)) accumulate over m in PSUM.
    PV lags the sim/exp chain by THREE z-steps (the chain latency spans
    ~2.5 steps; a shorter lag head-of-line blocks the PE queue).
  - out projection streamed from HBM as 32 column-tiles [8x128 rows x 512
    cols]: each tile is consumed by exactly ONE partial-projection group,
    so a pool slot recycles every z-step and the DMA engines stream w_out
    continuously instead of bursting at pair boundaries.  Final pair's
    partials run in a dense tail with per-block stores overlapping the
    matmuls.
"""

import sys

import numpy as np

for _p in ("/opt/trn_rl_repo",):
    if _p not in sys.path:
        sys.path.insert(0, _p)

import os  # noqa: E402

import ml_dtypes  # noqa: E402

USE_POOL = os.environ.get("K_POOL", "0") == "1"

B, N, M, D = 8, 128, 8, 512
H, DH = 8, 64
INNER = H * DH          # 512
T = N * M               # 1024 tokens per batch element
CD = INNER * M          # 4096 contraction dim of out projection
NCORES = 8

BF16 = ml_dtypes.bfloat16

_CACHE = {}


def build_nc():
    import concourse.bass as bass
    import concourse.mybir as mybir
    import concourse.tile as tile
    from concourse import bacc

    fp32 = mybir.dt.float32
    bf16 = mybir.dt.bfloat16

    nc = bacc.Bacc(trn_type="TRN2", target_bir_lowering=False, debug=False)

    xT = nc.dram_tensor("xT", (D, T), bf16, kind="ExternalInput").ap()
    w_q = nc.dram_tensor("w_q", (D, INNER), bf16, kind="ExternalInput").ap()
    w_k = nc.dram_tensor("w_k", (D, INNER), bf16, kind="ExternalInput").ap()
    w_v = nc.dram_tensor("w_v", (D, INNER), bf16, kind="ExternalInput").ap()
    w_out = nc.dram_tensor("w_out", (CD, CD), bf16, kind="ExternalInput").ap()
    y = nc.dram_tensor("y", (N, CD), fp32, kind="ExternalOutput").ap()

    KC = D // 128        # 4 contraction chunks for the projections
    PC = INNER // 128    # 4 partition chunks of qT/kT
    SCALE = DH ** -0.5
    NKC = CD // 128      # 32 contraction chunks of the out projection
    PV_LAG = 3

    with tile.TileContext(nc) as tc:
        with tc.tile_pool(name="persist", bufs=1) as persist:
            qT_sb = persist.tile([128, PC, T], bf16)
            kT_sb = persist.tile([128, PC, T], bf16)
            v_sb = persist.tile([128, M, INNER], bf16)
            ofT_sb = persist.tile([128, NKC, N], bf16)
            y_sb = persist.tile([128, CD], fp32)
            xT_sb = persist.tile([128, KC, T], bf16)
            wq_sb = persist.tile([128, KC, INNER], bf16)
            wk_sb = persist.tile([128, KC, INNER], bf16)
            wv_sb = persist.tile([128, KC, INNER], bf16)

            for kc in range(KC):
                nc.sync.dma_start(wq_sb[:, kc, :], w_q[kc * 128:(kc + 1) * 128, :])
                nc.sync.dma_start(xT_sb[:, kc, :], xT[kc * 128:(kc + 1) * 128, :])
            for kc in range(KC):
                nc.sync.dma_start(wk_sb[:, kc, :], w_k[kc * 128:(kc + 1) * 128, :])
            for kc in range(KC):
                nc.sync.dma_start(wv_sb[:, kc, :], w_v[kc * 128:(kc + 1) * 128, :])

            # w_out stream: half-column tiles [128 rows, 2048 cols] (512KB,
            # 4KB contiguous per partition -> cheap descriptors + big
            # packets).  A tile (kc, ch) is read by the four out-proj groups
            # nb = 4*ch .. 4*ch+3 of its pair, so slots recycle twice per
            # pair instead of once (smoother DMA).
            wo_pool = tc.alloc_tile_pool(name="wo_pool", bufs=26)
            wo_tiles = {}

            # ---- projections: qT/kT ((h dh) on partitions, tokens free), v ----
            with tc.tile_pool(name="proj_psum", bufs=4, space="PSUM") as proj_psum:
                for dst, w_sb in ((qT_sb, wq_sb), (kT_sb, wk_sb)):
                    for pc in range(PC):
                        for th in range(T // 512):
                            pj = proj_psum.tile([128, 512], fp32, name="pj", tag="pj")
                            for kc in range(KC):
                                nc.tensor.matmul(
                                    pj[:],
                                    w_sb[:, kc, pc * 128:(pc + 1) * 128],
                                    xT_sb[:, kc, th * 512:(th + 1) * 512],
                                    start=(kc == 0),
                                    stop=(kc == KC - 1),
                                )
                            nc.scalar.copy(dst[:, pc, th * 512:(th + 1) * 512], pj[:])
                for m in range(M):
                    pj = proj_psum.tile([128, 512], fp32, name="pj", tag="pj")
                    for kc in range(KC):
                        nc.tensor.matmul(
                            pj[:],
                            xT_sb[:, kc, m * 128:(m + 1) * 128],
                            wv_sb[:, kc, :],
                            start=(kc == 0),
                            stop=(kc == KC - 1),
                        )
                    if USE_POOL:
                        # store v/128: cancels the avg-pool's 1/128 exactly
                        nc.scalar.activation(
                            v_sb[:, m, :], pj[:],
                            mybir.ActivationFunctionType.Copy, scale=1.0 / 128,
                        )
                    else:
                        nc.scalar.copy(v_sb[:, m, :], pj[:])

            # w_out tile stream, issued after the projection trace so the
            # input loads own the DMA engines for the first few us.
            # Issue order == consumption order ((gp, ch) major).
            for gp in range(4):
                for ch in range(2):
                    for idx in range(M):
                        kc = 4 * idx + gp
                        wo_t = wo_pool.tile([128, 2048], bf16, name="wo_t", tag="wo")
                        nc.sync.dma_start(
                            wo_t[:],
                            w_out[kc * 128:(kc + 1) * 128,
                                  ch * 2048:(ch + 1) * 2048],
                        )
                        wo_tiles[(kc, ch)] = wo_t

            # ---- attention ----
            # Heads are processed in pairs (2g, 2g+1).  The two heads' sim
            # matmuls use K row-groups 0-63 / 64-127 and their PV matmuls use
            # output col-groups 0-63 / 64-127, so interleaving them lets the
            # PE array run both concurrently.
            with (
                tc.tile_pool(name="sim_psum", bufs=2, space="PSUM") as sim_psum,
                tc.tile_pool(name="pv_psum", bufs=1, space="PSUM") as pv_psum,
                tc.tile_pool(name="yp_psum", bufs=2, space="PSUM") as yp_psum,
                tc.tile_pool(name="p_pool", bufs=4) as p_pool,
                tc.tile_pool(name="vt_pool", bufs=5) as vt_pool,
                tc.tile_pool(name="stat_pool", bufs=10) as stat_pool,
            ):
                def emit_y_mms(gp, nb, pool=None):
                    # partial out-projection matmuls: pair gp's 8 contraction
                    # chunks into output columns [nb*512, (nb+1)*512).
                    yp = (pool or yp_psum).tile([128, 512], fp32, name="yp", tag="yp")
                    ch, co = nb // 4, (nb % 4) * 512
                    for idx in range(M):
                        kc = 4 * idx + gp
                        nc.tensor.matmul(
                            yp[:],
                            ofT_sb[:, kc, :],
                            wo_tiles[(kc, ch)][:, co:co + 512],
                            start=(idx == 0),
                            stop=(idx == M - 1),
                        )
                    return yp

                def emit_y_add(gp, nb, yp):
                    # VectorE accumulate, traced after the z-step's own DVE
                    # work to keep the DVE queue acyclic.
                    ysl = y_sb[:, nb * 512:(nb + 1) * 512]
                    if gp == 0:
                        nc.vector.tensor_copy(ysl, yp[:])
                    else:
                        nc.vector.tensor_tensor(
                            ysl, yp[:], ysl, op=mybir.AluOpType.add,
                        )

                for g in range(H // 2):  # head pairs
                    opv = pv_psum.tile([128, M * 128], fp32, name="opv", tag="opv")
                    pv_queue = []

                    def emit_pv(zz, p_z, vt_z, opv=opv):
                        # one accumulation group per head per z-region (groups
                        # in a PSUM zero region must not interleave start/stop)
                        for hh in range(2):
                            for m in range(M):
                                nc.tensor.matmul(
                                    opv[hh * 64:hh * 64 + 64, bass.ts(zz, 128)],
                                    vt_z[:, m, hh, :],
                                    p_z[:, hh * T + m * 128:hh * T + (m + 1) * 128],
                                    start=(m == 0),
                                    stop=(m == M - 1),
                                )
                    hc = g
                    qh = (qT_sb[0:64, hc, :], qT_sb[64:128, hc, :])
                    kh = (kT_sb[0:64, hc, :], kT_sb[64:128, hc, :])
                    for z in range(M):
                        # S^T_z per head: keys (z,j) on partitions, (m,i)
                        # free.  One PSUM tile per HEAD (both token halves):
                        # its exp is the first Scalar op of the step, so the
                        # tile recycles early and both heads' tiles are ready
                        # at step start -- the head-pair matmuls then stay
                        # interleaved in the PE queue and overlap on distinct
                        # row groups.
                        p_sb = p_pool.tile([128, 2 * T], bf16, name="p_sb", tag="p")
                        ps = [
                            sim_psum.tile([128, T], fp32, name=f"ps{hh}", tag="ps")
                            for hh in range(2)
                        ]
                        for th in range(T // 512):
                            for hh in range(2):
                                nc.tensor.matmul(
                                    ps[hh][:, bass.ts(th, 512)],
                                    kh[hh][:, bass.ts(z, 128)],
                                    qh[hh][:, bass.ts(th, 512)],
                                    start=True, stop=True,
                                )
                        # PV for z-PV_LAG (its exp/stats/vt chain is
                        # complete) goes right after sim, so ready work never
                        # queues behind out-proj matmuls that may wait on the
                        # w_out stream.
                        if len(pv_queue) >= PV_LAG:
                            emit_pv(*pv_queue.pop(0))
                        yp_fill = emit_y_mms(g - 1, z) if g > 0 else None
                        for hh in range(2):
                            nc.scalar.activation(
                                p_sb[:, hh * T:(hh + 1) * T], ps[hh][:],
                                mybir.ActivationFunctionType.Exp, scale=SCALE,
                            )
                        # L[j, (h, m)] = sum_i P^T[j, (h, m, i)] on VectorE.
                        # USE_POOL: avg-pool instead of tensor_reduce (the
                        # 1/128 is pre-folded into v at projection time).
                        lsum = stat_pool.tile([128, 2 * M], fp32, name="lsum", tag="ls")
                        pv3 = p_sb[:].rearrange("p (hm i) -> p hm i", i=128)
                        if USE_POOL:
                            nc.vector.pool_avg(lsum[:], pv3)
                        else:
                            nc.vector.tensor_reduce(
                                lsum[:], pv3,
                                axis=mybir.AxisListType.X, op=mybir.AluOpType.add,
                            )
                        linv = stat_pool.tile([128, 2 * M], fp32, name="linv", tag="li")
                        nc.vector.reciprocal(linv[:], lsum[:])
                        if yp_fill is not None:
                            emit_y_add(g - 1, z, yp_fill)
                        # vt[j, m, h, d] = v[j, m, (pair cols)] * Linv[j, (h, m)]
                        vt = vt_pool.tile([128, M, 2, DH], bf16, name="vt", tag="vt")
                        nc.gpsimd.tensor_tensor(
                            vt[:],
                            v_sb[:, :, g * 128:(g + 1) * 128].rearrange(
                                "p m (h d) -> p m h d", h=2
                            ),
                            linv[:].rearrange("p (h m) -> p m h", h=2)
                            .unsqueeze(3).broadcast_to((128, M, 2, DH)),
                            op=mybir.AluOpType.mult,
                        )
                        pv_queue.append((z, p_sb, vt))
                    for pv in pv_queue:  # flush the lagged z's of the pair
                        emit_pv(*pv)
                    # opv -> ofT split across Scalar/Vector so neither engine
                    # adds a full ~1.1us serial bubble at the pair boundary.
                    nc.scalar.copy(
                        ofT_sb[:, g:g + 16:4, :],
                        opv[:, 0:512].rearrange("p (z i) -> p z i", i=128),
                    )
                    nc.vector.tensor_copy(
                        ofT_sb[:, g + 16::4, :],
                        opv[:, 512:].rearrange("p (z i) -> p z i", i=128),
                    )

            # ---- last pair's out-projection partials + store ----
            with tc.tile_pool(name="ylast_psum", bufs=2, space="PSUM") as yp_psum2:
                for nb in range(CD // 512):
                    yp = emit_y_mms(3, nb, pool=yp_psum2)
                    ysl = y_sb[:, nb * 512:(nb + 1) * 512]
                    nc.vector.tensor_tensor(ysl, yp[:], ysl, op=mybir.AluOpType.add)
                    nc.sync.dma_start(y[:, nb * 512:(nb + 1) * 512], ysl)
            wo_pool.release()

    nc.compile()
    return nc


def _get_nc():
    if "nc" not in _CACHE:
        _CACHE["nc"] = build_nc()
    return _CACHE["nc"]


def _host_prep(x, w_q, w_kv, w_out):
    w_k = np.ascontiguousarray(w_kv[:, :INNER]).astype(BF16)
    w_v = np.ascontiguousarray(w_kv[:, INNER:]).astype(BF16)
    wq16 = np.ascontiguousarray(w_q).astype(BF16)
    wo16 = np.ascontiguousarray(w_out).astype(BF16)
    in_maps = []
    for b in range(B):
        # tokens modality-major: (M, N, D) -> (T, D); transpose to (D, T)
        xb = x[b].transpose(1, 0, 2).reshape(T, D)
        xT = np.ascontiguousarray(xb.T).astype(BF16)
        in_maps.append(
            {"xT": xT, "w_q": wq16, "w_k": w_k, "w_v": w_v, "w_out": wo16}
        )
    return in_maps


def kernel(x, w_q, w_kv, w_out, b_out):
    from concourse.bass_utils import run_bass_kernel_spmd

    nc = _get_nc()
    in_maps = _host_prep(
        np.asarray(x, np.float32),
        np.asarray(w_q, np.float32),
        np.asarray(w_kv, np.float32),
        np.asarray(w_out, np.float32),
    )
    res = run_bass_kernel_spmd(nc, in_maps, core_ids=list(range(NCORES)))
    ys = np.stack([res.results[c]["y"] for c in range(NCORES)], axis=0)
    ys = ys + np.asarray(b_out, np.float32)[None, None, :]
    return ys.reshape(B, N, M, D).astype(np.float32)
